# revision 10
# baseline (speedup 1.0000x reference)
"""Trainium2 distributed kernel for ABRLovaszCELoss (8 NeuronCores).

Strategy
--------
Data-parallel over (batch, row-half): core i handles batch b=i//2, fine rows
[192*(i%2), 192*(i%2)+192) of the 384x384 target grid (73728 pixels/core).

Per core, fully on-device:
- bilinear align_corners upsample 96->384 of all 19 logit channels
  (processing order head1:3, head2:2, head0:7, dsn:7) as two PE matmuls
  per channel (contract coarse-rows, then coarse-cols; transposed dataflow
  avoids any on-chip transpose). Pixel layout [128 part = X%128,
  576 free = 192*(X//128) + fy].
- softmax per head: exp fused into the PSUM->SBUF copy on the scalar
  engine; channel sums + reciprocal + scale + x = [tgt==c]-p on vector.
- cross-entropy: sum(ln S) - sum_c sum(z_c * [tgt==c]) via fused
  accumulate passes (no per-pixel gather).
- Lovasz-Softmax per shard via exact relu tail-integrals instead of a
  sort: with x = [tgt==c] - p_c,
     TF_j = sum relu(x - t_j)  = int_{t_j}^1 F(t) dt   (fg errors)
     TB_j = sum relu(-x - t_j) = int_{t_j}^1 B(t) dt   (bg errors)
  exact per-segment integrals IB/IF by differencing, and
     L_c ~= sum_j (IF_j + IB_j) / (n_c + K*IB_j)
  (Jaccard curve with segment-mean union).  The relu passes are fused
  compare+accumulate instructions split across the scalar and vector
  engines; per-shard Lovasz averaged over 8 shards differs from the
  batch-global reference by ~1e-6, the K-segment quadrature by ~2e-4.
- tiny per-class finale on one partition (interleaved across classes to
  dodge small-tile RAW hazards), then a single-scalar AllReduce.
"""

import numpy as np

import concourse.bass as bass
import concourse.mybir as mybir
from concourse.bass_utils import run_bass_kernel_spmd

F32 = mybir.dt.float32
AF = mybir.ActivationFunctionType
OP = mybir.AluOpType

NCH = 19
# channel order: head1 (3), head2 (2), head0 (7), dsn (7)
H1 = list(range(0, 3))
H2 = list(range(3, 5))
H0 = list(range(5, 12))
HD = list(range(12, 19))
K0 = 4
K12 = 8
P_GLOBAL = 4 * 384 * 384

# acc_sb column layout: class i base=40*i: TB at +0..K-1 (pad +K=0),
# TF at +20..20+K-1 (pad), n_c at +38, lovasz contrib at +39.
ACC_W = 512
COL_LNS0 = 480
COL_ZFG0 = 481
COL_LNSD = 488
COL_ZFGD = 489

# 12 lovasz classes: (head tag, class-in-head, K, acc base col)
CLASSES = (
    [("x0", c, K0, 40 * c) for c in range(7)]
    + [("x1", c, K12, 40 * (7 + c)) for c in range(3)]
    + [("x2", c, K12, 40 * (10 + c)) for c in range(2)]
)

DEBUG = False


def build_kernel():
    nc = bass.Bass()

    # const APs for activation bias values (-t_j thresholds)
    thr_vals = sorted(
        {-float(j) / K0 for j in range(1, K0)} | {-float(j) / K12 for j in range(1, K12)}
    )
    for i, val in enumerate(thr_vals):
        t = nc.alloc_sbuf_tensor(f"const-thr-{i}", [128, 1], F32)
        nc.gpsimd.memset(t.ap(), val)
        nc.const_aps.aps[(F32, val)] = t.ap()
    nc.all_engine_barrier()

    p_preds = nc.declare_dram_parameter("preds_all", [49, NCH * 96], F32, isOutput=False)
    p_uyt = nc.declare_dram_parameter("uyt", [49, 192], F32, isOutput=False)
    p_ux = nc.declare_dram_parameter("ux", [96, 384], F32, isOutput=False)
    p_tgt = nc.declare_dram_parameter("tgts", [128, 3 * 576], F32, isOutput=False)
    p_wv = nc.declare_dram_parameter("wvec", [1, ACC_W], F32, isOutput=False)
    p_out = nc.declare_dram_parameter("out", [1, 128], F32, isOutput=True)

    dbg = {}
    if DEBUG:
        for nm, shp in [
            ("d_x0", [128, 7 * 576]), ("d_s", [128, 4 * 576]),
            ("d_acc", [128, ACC_W]), ("d_rst", [1, ACC_W]),
        ]:
            dbg[nm] = nc.declare_dram_parameter(nm, shp, F32, isOutput=True)

    cc_in = nc.dram_tensor("cc_in", [1, 128], F32)
    cc_out = nc.dram_tensor("cc_out", [1, 128], F32, addr_space="Shared")
    core_ids = list(range(8))

    from contextlib import ExitStack
    with ExitStack() as es:
        def sb(name, shape, dtype=F32):
            return es.enter_context(nc.sbuf_tensor(name, shape, dtype))

        preds_sb = sb("preds_sb", [49, NCH * 96])
        uyt_sb = sb("uyt_sb", [49, 192])
        ux_sb = sb("ux_sb", [96, 384])
        tf_sb = sb("tf_sb", [128, 3 * 576])  # f32 targets: head0, head1, head2
        wv_sb = sb("wv_sb", [1, ACC_W])
        t1_sb = sb("t1_sb", [96, NCH * 192])
        z0_sb = sb("z0_sb", [128, 7 * 576])
        zd_sb = sb("zd_sb", [128, 7 * 576])
        x0_sb = sb("x0_sb", [128, 7 * 576])
        xd_sb = sb("xd_sb", [128, 7 * 576])
        x1_sb = sb("x1_sb", [128, 3 * 576])
        x2_sb = sb("x2_sb", [128, 2 * 576])
        s_sb = sb("s_sb", [128, 4 * 576])   # S0, Sd, S1, S2
        r_sb = sb("r_sb", [128, 3 * 576])   # R0, R1, R2
        onesw_sb = sb("onesw_sb", [128, 576])
        zero_sb = sb("zero_sb", [128, 576])
        onescol_sb = sb("onescol_sb", [128, 1])
        junk_v = sb("junk_v", [128, 576])
        junk_s = sb("junk_s", [128, 576])
        acc_sb = sb("acc_sb", [128, ACC_W])
        rst_sb = sb("rst_sb", [1, ACC_W])
        tmpA_sb = sb("tmpA_sb", [1, 256])
        tmpB_sb = sb("tmpB_sb", [1, 256])
        tmpC_sb = sb("tmpC_sb", [1, 256])
        tmpD_sb = sb("tmpD_sb", [1, 256])
        tmpE_sb = sb("tmpE_sb", [1, 256])
        tmpw_sb = sb("tmpw_sb", [1, ACC_W])
        partial_sb = sb("partial_sb", [1, 128])

        ps1 = [es.enter_context(nc.psum_tensor(f"ps1{i}", [96, 192], F32)) for i in range(2)]
        ps2 = [es.enter_context(nc.psum_tensor(f"ps2{i}", [128, 192], F32)) for i in range(4)]
        psR = es.enter_context(nc.psum_tensor("psR", [1, ACC_W], F32))

        dma_sem = es.enter_context(nc.semaphore("dma_sem"))
        mm_sem = es.enter_context(nc.semaphore("mm_sem"))
        cp_sem = es.enter_context(nc.semaphore("cp_sem"))
        vx_sem = es.enter_context(nc.semaphore("vx_sem"))
        hist_sem = es.enter_context(nc.semaphore("hist_sem"))
        fin_sem = es.enter_context(nc.semaphore("fin_sem"))
        cdma_sem = es.enter_context(nc.semaphore("cdma_sem"))
        cc_sem = es.enter_context(nc.semaphore("cc_sem"))

        def thr(K):
            return [float(j) / K for j in range(K)]

        # head copy-completion cp_sem thresholds (stage2 is channel-outer)
        CP_H1 = 19 + 3 * 3
        CP_H2 = 19 + 5 * 3
        CP_H0 = 19 + 12 * 3
        CP_HD = 19 + 19 * 3  # = 76

        with nc.Block() as block:

            @block.sync
            def _(sync):
                sync.dma_start(out=preds_sb[:, :], in_=p_preds[:, :]).then_inc(dma_sem, 16)
                sync.dma_start(out=uyt_sb[:, :], in_=p_uyt[:, :]).then_inc(dma_sem, 16)
                sync.dma_start(out=tf_sb[:, :], in_=p_tgt[:, :]).then_inc(dma_sem, 16)
                sync.dma_start(out=ux_sb[:, :], in_=p_ux[:, :]).then_inc(dma_sem, 16)
                sync.dma_start(out=wv_sb[:, :], in_=p_wv[:, :]).then_inc(dma_sem, 16)

            @block.tensor
            def _(tensor):
                tensor.wait_ge(dma_sem, 32)
                # stage 1: out1_c[cx, fy] = z_c^T @ uyt
                for c in range(NCH):
                    if c >= 2:
                        tensor.wait_ge(cp_sem, c - 1)
                    tensor.matmul(
                        ps1[c % 2][0:96, 0:192],
                        preds_sb[0:49, 96 * c : 96 * (c + 1)],
                        uyt_sb[0:49, 0:192],
                        start=True, stop=True,
                    ).then_inc(mm_sem)
                tensor.wait_ge(dma_sem, 64)
                # stage 2 (channel-outer): z_c[X, fy] = ux_k^T @ t1_c
                for c in range(NCH):
                    for k in range(3):
                        idx = 3 * c + k
                        if idx >= 4:
                            tensor.wait_ge(cp_sem, 19 + idx - 3)
                        tensor.matmul(
                            ps2[idx % 4][0:128, 0:192],
                            ux_sb[0:96, 128 * k : 128 * (k + 1)],
                            t1_sb[0:96, 192 * c : 192 * (c + 1)],
                            start=True, stop=True,
                        ).then_inc(mm_sem)
                # partition reduction of the accumulator stack
                tensor.wait_ge(hist_sem, 2)
                tensor.matmul(
                    psR[0:1, 0:ACC_W],
                    onescol_sb[0:128, 0:1],
                    acc_sb[0:128, 0:ACC_W],
                    start=True, stop=True,
                ).then_inc(mm_sem)

            @block.scalar
            def _(scalar):
                # stage 1 copies PSUM -> t1
                for c in range(NCH):
                    scalar.wait_ge(mm_sem, c + 1)
                    scalar.activation(
                        t1_sb[0:96, 192 * c : 192 * (c + 1)],
                        ps1[c % 2][0:96, 0:192], AF.Copy,
                    ).then_inc(cp_sem)
                # stage 2 copies: exp for all; raw z also for head0/dsn
                for c in range(NCH):
                    for k in range(3):
                        idx = 3 * c + k
                        scalar.wait_ge(mm_sem, 19 + idx + 1)
                        src = ps2[idx % 4][0:128, 0:192]
                        if c in H1:
                            ci = c - H1[0]
                            dst = slice(576 * ci + 192 * k, 576 * ci + 192 * (k + 1))
                            scalar.activation(x1_sb[:, dst], src, AF.Exp).then_inc(cp_sem)
                        elif c in H2:
                            ci = c - H2[0]
                            dst = slice(576 * ci + 192 * k, 576 * ci + 192 * (k + 1))
                            scalar.activation(x2_sb[:, dst], src, AF.Exp).then_inc(cp_sem)
                        elif c in H0:
                            ci = c - H0[0]
                            dst = slice(576 * ci + 192 * k, 576 * ci + 192 * (k + 1))
                            scalar.activation(z0_sb[:, dst], src, AF.Copy)
                            scalar.activation(x0_sb[:, dst], src, AF.Exp).then_inc(cp_sem)
                        else:
                            ci = c - HD[0]
                            dst = slice(576 * ci + 192 * k, 576 * ci + 192 * (k + 1))
                            scalar.activation(zd_sb[:, dst], src, AF.Copy)
                            scalar.activation(xd_sb[:, dst], src, AF.Exp).then_inc(cp_sem)
                # scalar-owned hist: head2 TB, lnS0, head0 TB+TF, lnSd
                scalar.wait_ge(vx_sem, 2)  # x2 ready
                for c in range(2):
                    base = 40 * (10 + c)
                    xs = x2_sb[:, 576 * c : 576 * (c + 1)]
                    for j, t in enumerate(thr(K12)):
                        scalar.activation(
                            junk_s[:, :], xs, AF.Relu, bias=-t, scale=-1.0,
                            accum_out=acc_sb[:, base + j : base + j + 1],
                        )
                scalar.wait_ge(vx_sem, 3)  # S0 ready
                scalar.activation(
                    junk_s[:, :], s_sb[:, 0:576], AF.Ln,
                    accum_out=acc_sb[:, COL_LNS0 : COL_LNS0 + 1],
                )
                scalar.wait_ge(vx_sem, 4)  # x0 ready
                for c in range(7):
                    base = 40 * c
                    xs = x0_sb[:, 576 * c : 576 * (c + 1)]
                    for j, t in enumerate(thr(K0)):
                        scalar.activation(
                            junk_s[:, :], xs, AF.Relu, bias=-t, scale=-1.0,
                            accum_out=acc_sb[:, base + j : base + j + 1],
                        )
                    for j, t in enumerate(thr(K0)):
                        scalar.activation(
                            junk_s[:, :], xs, AF.Relu, bias=-t, scale=1.0,
                            accum_out=acc_sb[:, base + 20 + j : base + 21 + j],
                        )
                scalar.wait_ge(vx_sem, 5)  # Sd ready
                scalar.activation(
                    junk_s[:, :], s_sb[:, 576:1152], AF.Ln,
                    accum_out=acc_sb[:, COL_LNSD : COL_LNSD + 1],
                ).then_inc(hist_sem)

            @block.vector
            def _(vector):
                vector.memset(acc_sb[:, :], 0.0)
                vector.memset(onesw_sb[:, :], 1.0)
                vector.memset(zero_sb[:, :], 0.0)
                vector.memset(onescol_sb[:, :], 1.0)
                vector.memset(partial_sb[:, :], 0.0)
                vector.wait_ge(dma_sem, 48)
                # n_c counts (need only targets)
                tfo = {"x0": 0, "x1": 1, "x2": 2}
                for (xk, c, K, base) in CLASSES:
                    h = tfo[xk]
                    vector.scalar_tensor_tensor(
                        junk_v[:, :], tf_sb[:, 576 * h : 576 * (h + 1)], float(c),
                        onesw_sb[:, :], OP.is_equal, OP.mult,
                        accum_out=acc_sb[:, base + 38 : base + 39],
                    )
                # ---- head1 softmax + x ----
                vector.wait_ge(cp_sem, CP_H1)
                vector.tensor_add(s_sb[:, 1152:1728], x1_sb[:, 0:576], x1_sb[:, 576:1152])
                vector.tensor_add(s_sb[:, 1152:1728], s_sb[:, 1152:1728], x1_sb[:, 1152:1728])
                vector.reciprocal(r_sb[:, 576:1152], s_sb[:, 1152:1728])
                ins = None
                for c in range(3):
                    cs = slice(576 * c, 576 * (c + 1))
                    vector.tensor_mul(x1_sb[:, cs], x1_sb[:, cs], r_sb[:, 576:1152])
                    ins = vector.scalar_tensor_tensor(
                        x1_sb[:, cs], tf_sb[:, 576:1152], float(c), x1_sb[:, cs],
                        OP.is_equal, OP.subtract,
                    )
                ins.then_inc(vx_sem)  # vx=1: x1
                # ---- head2 ----
                vector.wait_ge(cp_sem, CP_H2)
                vector.tensor_add(s_sb[:, 1728:2304], x2_sb[:, 0:576], x2_sb[:, 576:1152])
                vector.reciprocal(r_sb[:, 1152:1728], s_sb[:, 1728:2304])
                for c in range(2):
                    cs = slice(576 * c, 576 * (c + 1))
                    vector.tensor_mul(x2_sb[:, cs], x2_sb[:, cs], r_sb[:, 1152:1728])
                    ins = vector.scalar_tensor_tensor(
                        x2_sb[:, cs], tf_sb[:, 1152:1728], float(c), x2_sb[:, cs],
                        OP.is_equal, OP.subtract,
                    )
                ins.then_inc(vx_sem)  # vx=2: x2
                # ---- head0 ----
                vector.wait_ge(cp_sem, CP_H0)
                vector.tensor_add(s_sb[:, 0:576], x0_sb[:, 0:576], x0_sb[:, 576:1152])
                for c in range(2, 7):
                    ins = vector.tensor_add(
                        s_sb[:, 0:576], s_sb[:, 0:576], x0_sb[:, 576 * c : 576 * (c + 1)]
                    )
                ins.then_inc(vx_sem)  # vx=3: S0
                vector.reciprocal(r_sb[:, 0:576], s_sb[:, 0:576])
                for c in range(7):
                    cs = slice(576 * c, 576 * (c + 1))
                    vector.tensor_mul(x0_sb[:, cs], x0_sb[:, cs], r_sb[:, 0:576])
                    ins = vector.scalar_tensor_tensor(
                        x0_sb[:, cs], tf_sb[:, 0:576], float(c), x0_sb[:, cs],
                        OP.is_equal, OP.subtract,
                    )
                ins.then_inc(vx_sem)  # vx=4: x0
                # ---- dsn S ----
                vector.wait_ge(cp_sem, CP_HD)
                vector.tensor_add(s_sb[:, 576:1152], xd_sb[:, 0:576], xd_sb[:, 576:1152])
                for c in range(2, 7):
                    ins = vector.tensor_add(
                        s_sb[:, 576:1152], s_sb[:, 576:1152], xd_sb[:, 576 * c : 576 * (c + 1)]
                    )
                ins.then_inc(vx_sem)  # vx=5: Sd
                # CE z*fg accumulations
                for c in range(7):
                    vector.scalar_tensor_tensor(
                        junk_v[:, :], tf_sb[:, 0:576], float(c),
                        z0_sb[:, 576 * c : 576 * (c + 1)], OP.is_equal, OP.mult,
                        accum_out=acc_sb[:, COL_ZFG0 + c : COL_ZFG0 + c + 1],
                    )
                for c in range(7):
                    vector.scalar_tensor_tensor(
                        junk_v[:, :], tf_sb[:, 0:576], float(c),
                        zd_sb[:, 576 * c : 576 * (c + 1)], OP.is_equal, OP.mult,
                        accum_out=acc_sb[:, COL_ZFGD + c : COL_ZFGD + c + 1],
                    )
                # vector-owned hist: head1 TB (as -TB via min) + TF, head2 TF
                last = None
                for c in range(3):
                    base = 40 * (7 + c)
                    xs = x1_sb[:, 576 * c : 576 * (c + 1)]
                    for j, t in enumerate(thr(K12)):
                        # min(x + t, 0) accumulates -TB_j
                        last = vector.scalar_tensor_tensor(
                            junk_v[:, :], xs, t, zero_sb[:, :], OP.add, OP.min,
                            accum_out=acc_sb[:, base + j : base + j + 1],
                        )
                    for j, t in enumerate(thr(K12)):
                        last = vector.scalar_tensor_tensor(
                            junk_v[:, :], xs, t, zero_sb[:, :], OP.subtract, OP.max,
                            accum_out=acc_sb[:, base + 20 + j : base + 21 + j],
                        )
                for c in range(2):
                    base = 40 * (10 + c)
                    xs = x2_sb[:, 576 * c : 576 * (c + 1)]
                    for j, t in enumerate(thr(K12)):
                        last = vector.scalar_tensor_tensor(
                            junk_v[:, :], xs, t, zero_sb[:, :], OP.subtract, OP.max,
                            accum_out=acc_sb[:, base + 20 + j : base + 21 + j],
                        )
                last.then_inc(hist_sem)
                # ---- finale (interleaved to dodge small-tile RAW hazards) ----
                vector.wait_ge(mm_sem, 77)
                vector.tensor_copy(rst_sb[0:1, :], psR[0:1, :])
                vector.memset(junk_v[:, :], 0.0)  # spacer
                for i, (xk, c, K, base) in enumerate(CLASSES):
                    if xk == "x1":  # TB cols hold -TB: flip the difference
                        vector.tensor_sub(
                            tmpA_sb[0:1, 16 * i : 16 * i + K],
                            rst_sb[0:1, base + 1 : base + K + 1],
                            rst_sb[0:1, base : base + K],
                        )
                    else:
                        vector.tensor_sub(
                            tmpA_sb[0:1, 16 * i : 16 * i + K],
                            rst_sb[0:1, base : base + K],
                            rst_sb[0:1, base + 1 : base + K + 1],
                        )
                for i, (xk, c, K, base) in enumerate(CLASSES):
                    vector.tensor_scalar(
                        tmpB_sb[0:1, 16 * i : 16 * i + K],
                        tmpA_sb[0:1, 16 * i : 16 * i + K],
                        float(K), rst_sb[0:1, base + 38 : base + 39],
                        OP.mult, OP.add,
                    )
                for i, (xk, c, K, base) in enumerate(CLASSES):
                    vector.reciprocal(
                        tmpC_sb[0:1, 16 * i : 16 * i + K],
                        tmpB_sb[0:1, 16 * i : 16 * i + K],
                    )
                for i, (xk, c, K, base) in enumerate(CLASSES):
                    vector.tensor_sub(
                        tmpD_sb[0:1, 16 * i : 16 * i + K],
                        rst_sb[0:1, base + 20 : base + 20 + K],
                        rst_sb[0:1, base + 21 : base + 21 + K],
                    )
                for i, (xk, c, K, base) in enumerate(CLASSES):
                    vector.tensor_add(
                        tmpD_sb[0:1, 16 * i : 16 * i + K],
                        tmpD_sb[0:1, 16 * i : 16 * i + K],
                        tmpA_sb[0:1, 16 * i : 16 * i + K],
                    )
                for i, (xk, c, K, base) in enumerate(CLASSES):
                    vector.scalar_tensor_tensor(
                        tmpE_sb[0:1, 16 * i : 16 * i + K],
                        tmpD_sb[0:1, 16 * i : 16 * i + K],
                        1.0, tmpC_sb[0:1, 16 * i : 16 * i + K],
                        OP.mult, OP.mult,
                        accum_out=rst_sb[0:1, base + 39 : base + 40],
                    )
                vector.memset(junk_v[:, :], 0.0)  # spacer
                vector.tensor_mul(tmpw_sb[0:1, :], rst_sb[0:1, :], wv_sb[0:1, :])
                vector.memset(junk_v[:, :], 0.0)  # spacer
                vector.tensor_reduce(
                    partial_sb[0:1, 0:1], tmpw_sb[0:1, :],
                    mybir.AxisListType.X, OP.add,
                ).then_inc(fin_sem)

            @block.gpsimd
            def _(gpsimd):
                gpsimd.wait_ge(fin_sem, 1)
                gpsimd.dma_start(out=cc_in[:, :], in_=partial_sb[0:1, :]).then_inc(cdma_sem, 16)
                gpsimd.wait_ge(cdma_sem, 16)
                gpsimd.collective_compute(
                    "AllReduce", OP.add, replica_groups=[core_ids],
                    ins=[cc_in[:, :]], outs=[cc_out[:, :]],
                ).then_inc(cc_sem)
                gpsimd.wait_ge(cc_sem, 1)
                gpsimd.dma_start(out=p_out[:, :], in_=cc_out[:, :]).then_inc(cdma_sem, 16)
                gpsimd.wait_ge(cdma_sem, 32)
                if DEBUG:
                    n = 32
                    for name, t in [("d_x0", x0_sb), ("d_s", s_sb), ("d_acc", acc_sb), ("d_rst", rst_sb)]:
                        gpsimd.dma_start(out=dbg[name][:, :], in_=t[:, :]).then_inc(cdma_sem, 16)
                        n += 16
                        gpsimd.wait_ge(cdma_sem, n)

    return nc


# ---------------------------------------------------------------- host side --

def _interp_weights():
    s = np.linspace(np.float32(0.0), np.float32(95.0), 384).astype(np.float32)
    i0 = np.clip(np.floor(s).astype(np.int64), 0, 94)
    t = (s - i0).astype(np.float32)
    return i0, t


def _prep_core(inputs, core):
    b, half = core // 2, core % 2
    r0 = half * 192
    cy0 = 0 if half == 0 else 47
    i0, t = _interp_weights()

    uyt = np.zeros((49, 192), np.float32)
    for fy in range(192):
        f = r0 + fy
        uyt[i0[f] - cy0, fy] += np.float32(1.0) - t[f]
        uyt[i0[f] + 1 - cy0, fy] += t[f]

    ux = np.zeros((96, 384), np.float32)
    for X in range(384):
        ux[i0[X], X] += np.float32(1.0) - t[X]
        ux[i0[X] + 1, X] += t[X]

    # channel order: head1, head2, head0, dsn
    heads = [inputs["preds1"], inputs["preds2"], inputs["preds0"], inputs["preds_dsn"]]
    pa = np.zeros((49, NCH * 96), np.float32)
    idx = 0
    for arr in heads:
        for ch in range(arr.shape[1]):
            pa[:, idx * 96 : (idx + 1) * 96] = arr[b, ch, cy0 : cy0 + 49, :]
            idx += 1

    tg = np.zeros((128, 3 * 576), np.float32)
    for h, key in enumerate(["targets0", "targets1", "targets2"]):
        th = inputs[key][b, r0 : r0 + 192, :]  # [192, 384]
        tg[:, 576 * h : 576 * (h + 1)] = (
            th.reshape(192, 3, 128).transpose(2, 1, 0).reshape(128, 576)
        ).astype(np.float32)

    wv = np.zeros((1, ACC_W), np.float32)
    for (xk, c, K, base) in CLASSES:
        if xk == "x0":
            wv[0, base + 39] = (1.0 / 7.0) / 8.0
        elif xk == "x1":
            wv[0, base + 39] = (0.4 / 3.0) / 8.0
        else:
            wv[0, base + 39] = (0.4 / 2.0) / 8.0
    wv[0, COL_LNS0] = 1.0 / P_GLOBAL
    wv[0, COL_ZFG0 : COL_ZFG0 + 7] = -1.0 / P_GLOBAL
    wv[0, COL_LNSD] = 0.4 / P_GLOBAL
    wv[0, COL_ZFGD : COL_ZFGD + 7] = -0.4 / P_GLOBAL

    return {"preds_all": pa, "uyt": uyt, "ux": ux, "tgts": tg, "wvec": wv}


_NC_CACHE = None


def kernel(**inputs):
    global _NC_CACHE
    inputs = {k: np.asarray(v) for k, v in inputs.items()}
    if _NC_CACHE is None:
        _NC_CACHE = build_kernel()
    nc = _NC_CACHE
    in_maps = [_prep_core(inputs, core) for core in range(8)]
    res = run_bass_kernel_spmd(nc, in_maps, core_ids=list(range(8)))
    out = np.asarray(res.results[0]["out"], dtype=np.float32).reshape(-1)
    return np.asarray(out[0], dtype=np.float32)


# revision 12
# speedup vs baseline: 1.0122x; 1.0122x over previous
"""Trainium2 distributed kernel for ABRLovaszCELoss (8 NeuronCores).

Strategy
--------
Data-parallel over (batch, row-half): core i handles batch b=i//2, fine rows
[192*(i%2), 192*(i%2)+192) of the 384x384 target grid (73728 pixels/core).

Per core, fully on-device:
- bilinear align_corners upsample 96->384 of all 19 logit channels
  (processing order head1:3, head2:2, head0:7, dsn:7) as two PE matmuls
  per channel (contract coarse-rows, then coarse-cols; transposed dataflow
  avoids any on-chip transpose). Pixel layout [128 part = X%128,
  576 free = 192*(X//128) + fy].
- softmax per head: exp fused into the PSUM->SBUF copy on the scalar
  engine; channel sums + reciprocal + scale + x = [tgt==c]-p on vector.
- cross-entropy: sum(ln S) - sum_c sum(z_c * [tgt==c]) via fused
  accumulate passes (no per-pixel gather).
- Lovasz-Softmax per shard via exact relu tail-integrals instead of a
  sort: with x = [tgt==c] - p_c,
     TF_j = sum relu(x - t_j)  = int_{t_j}^1 F(t) dt   (fg errors)
     TB_j = sum relu(-x - t_j) = int_{t_j}^1 B(t) dt   (bg errors)
  exact per-segment integrals IB/IF by differencing, and
     L_c ~= sum_j (IF_j + IB_j) / (n_c + K*IB_j)
  (Jaccard curve with segment-mean union).  The relu passes are fused
  compare+accumulate instructions split across the scalar and vector
  engines; per-shard Lovasz averaged over 8 shards differs from the
  batch-global reference by ~1e-6, the K-segment quadrature by ~2e-4.
- tiny per-class finale on one partition (interleaved across classes to
  dodge small-tile RAW hazards), then a single-scalar AllReduce.
"""

import numpy as np

import concourse.bass as bass
import concourse.mybir as mybir
from concourse.bass_utils import run_bass_kernel_spmd

F32 = mybir.dt.float32
AF = mybir.ActivationFunctionType
OP = mybir.AluOpType

NCH = 19
# channel order: head1 (3), head2 (2), head0 (7), dsn (7)
H1 = list(range(0, 3))
H2 = list(range(3, 5))
H0 = list(range(5, 12))
HD = list(range(12, 19))
K0 = 4
K12 = 8
P_GLOBAL = 4 * 384 * 384

# acc_sb column layout: class i base=40*i: TB at +0..K-1 (pad +K=0),
# TF at +20..20+K-1 (pad), n_c at +38, lovasz contrib at +39.
ACC_W = 512
COL_LNS0 = 480
COL_ZFG0 = 481
COL_LNSD = 488
COL_ZFGD = 489

# 12 lovasz classes: (head tag, class-in-head, K, acc base col)
CLASSES = (
    [("x0", c, K0, 40 * c) for c in range(7)]
    + [("x1", c, K12, 40 * (7 + c)) for c in range(3)]
    + [("x2", c, K12, 40 * (10 + c)) for c in range(2)]
)

DEBUG = False


def build_kernel():
    nc = bass.Bass()

    # const APs for activation bias values (-t_j thresholds)
    thr_vals = sorted(
        {-float(j) / K0 for j in range(1, K0)} | {-float(j) / K12 for j in range(1, K12)}
    )
    for i, val in enumerate(thr_vals):
        t = nc.alloc_sbuf_tensor(f"const-thr-{i}", [128, 1], F32)
        nc.gpsimd.memset(t.ap(), val)
        nc.const_aps.aps[(F32, val)] = t.ap()
    nc.all_engine_barrier()

    p_preds = nc.declare_dram_parameter("preds_all", [49, NCH * 96], F32, isOutput=False)
    p_uyt = nc.declare_dram_parameter("uyt", [49, 192], F32, isOutput=False)
    p_ux = nc.declare_dram_parameter("ux", [96, 384], F32, isOutput=False)
    p_tgt = nc.declare_dram_parameter("tgts", [128, 3 * 576], F32, isOutput=False)
    p_wv = nc.declare_dram_parameter("wvec", [1, ACC_W], F32, isOutput=False)
    p_out = nc.declare_dram_parameter("out", [1, 128], F32, isOutput=True)

    dbg = {}
    if DEBUG:
        for nm, shp in [
            ("d_x0", [128, 7 * 576]), ("d_s", [128, 4 * 576]),
            ("d_acc", [128, ACC_W]), ("d_rst", [1, ACC_W]),
        ]:
            dbg[nm] = nc.declare_dram_parameter(nm, shp, F32, isOutput=True)

    cc_in = nc.dram_tensor("cc_in", [1, 128], F32)
    cc_out = nc.dram_tensor("cc_out", [1, 128], F32, addr_space="Shared")
    core_ids = list(range(8))

    from contextlib import ExitStack
    with ExitStack() as es:
        def sb(name, shape, dtype=F32):
            return es.enter_context(nc.sbuf_tensor(name, shape, dtype))

        preds_sb = sb("preds_sb", [49, NCH * 96])
        uyt_sb = sb("uyt_sb", [49, 192])
        ux_sb = sb("ux_sb", [96, 384])
        tf_sb = sb("tf_sb", [128, 3 * 576])  # f32 targets: head0, head1, head2
        wv_sb = sb("wv_sb", [1, ACC_W])
        t1_sb = sb("t1_sb", [96, NCH * 192])
        z0_sb = sb("z0_sb", [128, 7 * 576])
        zd_sb = sb("zd_sb", [128, 7 * 576])
        x0_sb = sb("x0_sb", [128, 7 * 576])
        xd_sb = sb("xd_sb", [128, 7 * 576])
        x1_sb = sb("x1_sb", [128, 3 * 576])
        x2_sb = sb("x2_sb", [128, 2 * 576])
        s_sb = sb("s_sb", [128, 4 * 576])   # S0, Sd, S1, S2
        r_sb = sb("r_sb", [128, 3 * 576])   # R0, R1, R2
        onesw_sb = sb("onesw_sb", [128, 576])
        zero_sb = sb("zero_sb", [128, 576])
        onescol_sb = sb("onescol_sb", [128, 1])
        junk_v = sb("junk_v", [128, 576])
        junk_s = sb("junk_s", [128, 576])
        acc_sb = sb("acc_sb", [128, ACC_W])
        rst_sb = sb("rst_sb", [1, ACC_W])
        tmpA_sb = sb("tmpA_sb", [1, 256])
        tmpB_sb = sb("tmpB_sb", [1, 256])
        tmpC_sb = sb("tmpC_sb", [1, 256])
        tmpD_sb = sb("tmpD_sb", [1, 256])
        tmpE_sb = sb("tmpE_sb", [1, 256])
        tmpw_sb = sb("tmpw_sb", [1, ACC_W])
        partial_sb = sb("partial_sb", [1, 128])

        ps1 = [es.enter_context(nc.psum_tensor(f"ps1{i}", [96, 192], F32)) for i in range(2)]
        ps2 = [es.enter_context(nc.psum_tensor(f"ps2{i}", [128, 192], F32)) for i in range(4)]
        psR = es.enter_context(nc.psum_tensor("psR", [1, ACC_W], F32))

        dma_sem = es.enter_context(nc.semaphore("dma_sem"))
        dmaP_sem = es.enter_context(nc.semaphore("dmaP_sem"))
        dmaU_sem = es.enter_context(nc.semaphore("dmaU_sem"))
        dmaT_sem = es.enter_context(nc.semaphore("dmaT_sem"))
        dmaX_sem = es.enter_context(nc.semaphore("dmaX_sem"))
        dmaW_sem = es.enter_context(nc.semaphore("dmaW_sem"))
        mm_sem = es.enter_context(nc.semaphore("mm_sem"))
        cp_sem = es.enter_context(nc.semaphore("cp_sem"))
        vx_sem = es.enter_context(nc.semaphore("vx_sem"))
        hist_sem = es.enter_context(nc.semaphore("hist_sem"))
        fin_sem = es.enter_context(nc.semaphore("fin_sem"))
        cdma_sem = es.enter_context(nc.semaphore("cdma_sem"))
        cc_sem = es.enter_context(nc.semaphore("cc_sem"))

        def thr(K):
            return [float(j) / K for j in range(K)]

        # head copy-completion cp_sem thresholds (stage2 is channel-outer)
        CP_H1 = 19 + 3 * 3
        CP_H2 = 19 + 5 * 3
        CP_H0 = 19 + 12 * 3
        CP_HD = 19 + 19 * 3  # = 76

        with nc.Block() as block:

            @block.sync
            def _(sync):
                sync.dma_start(out=preds_sb[:, :], in_=p_preds[:, :]).then_inc(dmaP_sem, 16)
                sync.dma_start(out=uyt_sb[:, :], in_=p_uyt[:, :]).then_inc(dmaU_sem, 16)
                sync.dma_start(out=tf_sb[:, :], in_=p_tgt[:, :]).then_inc(dmaT_sem, 16)
                sync.dma_start(out=ux_sb[:, :], in_=p_ux[:, :]).then_inc(dmaX_sem, 16)
                sync.dma_start(out=wv_sb[:, :], in_=p_wv[:, :]).then_inc(dmaW_sem, 16)

            @block.tensor
            def _(tensor):
                tensor.wait_ge(dmaP_sem, 16)
                tensor.wait_ge(dmaU_sem, 16)
                # stage 1: out1_c[cx, fy] = z_c^T @ uyt
                for c in range(NCH):
                    if c >= 2:
                        tensor.wait_ge(cp_sem, c - 1)
                    tensor.matmul(
                        ps1[c % 2][0:96, 0:192],
                        preds_sb[0:49, 96 * c : 96 * (c + 1)],
                        uyt_sb[0:49, 0:192],
                        start=True, stop=True,
                    ).then_inc(mm_sem)
                tensor.wait_ge(dmaX_sem, 16)
                # stage 2 (channel-outer): z_c[X, fy] = ux_k^T @ t1_c
                for c in range(NCH):
                    for k in range(3):
                        idx = 3 * c + k
                        if idx >= 4:
                            tensor.wait_ge(cp_sem, 19 + idx - 3)
                        tensor.matmul(
                            ps2[idx % 4][0:128, 0:192],
                            ux_sb[0:96, 128 * k : 128 * (k + 1)],
                            t1_sb[0:96, 192 * c : 192 * (c + 1)],
                            start=True, stop=True,
                        ).then_inc(mm_sem)
                # partition reduction of the accumulator stack
                tensor.wait_ge(hist_sem, 2)
                tensor.matmul(
                    psR[0:1, 0:ACC_W],
                    onescol_sb[0:128, 0:1],
                    acc_sb[0:128, 0:ACC_W],
                    start=True, stop=True,
                ).then_inc(mm_sem)

            @block.scalar
            def _(scalar):
                # stage 1 copies PSUM -> t1
                for c in range(NCH):
                    scalar.wait_ge(mm_sem, c + 1)
                    scalar.activation(
                        t1_sb[0:96, 192 * c : 192 * (c + 1)],
                        ps1[c % 2][0:96, 0:192], AF.Copy,
                    ).then_inc(cp_sem)
                # stage 2 copies: exp for all; raw z also for head0/dsn
                for c in range(NCH):
                    for k in range(3):
                        idx = 3 * c + k
                        scalar.wait_ge(mm_sem, 19 + idx + 1)
                        src = ps2[idx % 4][0:128, 0:192]
                        if c in H1:
                            ci = c - H1[0]
                            dst = slice(576 * ci + 192 * k, 576 * ci + 192 * (k + 1))
                            scalar.activation(x1_sb[:, dst], src, AF.Exp).then_inc(cp_sem)
                        elif c in H2:
                            ci = c - H2[0]
                            dst = slice(576 * ci + 192 * k, 576 * ci + 192 * (k + 1))
                            scalar.activation(x2_sb[:, dst], src, AF.Exp).then_inc(cp_sem)
                        elif c in H0:
                            ci = c - H0[0]
                            dst = slice(576 * ci + 192 * k, 576 * ci + 192 * (k + 1))
                            scalar.activation(z0_sb[:, dst], src, AF.Copy)
                            scalar.activation(x0_sb[:, dst], src, AF.Exp).then_inc(cp_sem)
                        else:
                            ci = c - HD[0]
                            dst = slice(576 * ci + 192 * k, 576 * ci + 192 * (k + 1))
                            scalar.activation(zd_sb[:, dst], src, AF.Copy)
                            scalar.activation(xd_sb[:, dst], src, AF.Exp).then_inc(cp_sem)
                # scalar-owned hist: head2 TB, lnS0, head0 TB+TF, lnSd
                scalar.wait_ge(vx_sem, 2)  # x2 ready
                for c in range(2):
                    base = 40 * (10 + c)
                    xs = x2_sb[:, 576 * c : 576 * (c + 1)]
                    for j, t in enumerate(thr(K12)):
                        scalar.activation(
                            junk_s[:, :], xs, AF.Relu, bias=-t, scale=-1.0,
                            accum_out=acc_sb[:, base + j : base + j + 1],
                        )
                scalar.wait_ge(vx_sem, 3)  # S0 ready
                scalar.activation(
                    junk_s[:, :], s_sb[:, 0:576], AF.Ln,
                    accum_out=acc_sb[:, COL_LNS0 : COL_LNS0 + 1],
                )
                scalar.wait_ge(vx_sem, 4)  # x0 ready
                for c in range(7):
                    base = 40 * c
                    xs = x0_sb[:, 576 * c : 576 * (c + 1)]
                    for j, t in enumerate(thr(K0)):
                        scalar.activation(
                            junk_s[:, :], xs, AF.Relu, bias=-t, scale=-1.0,
                            accum_out=acc_sb[:, base + j : base + j + 1],
                        )
                    for j, t in enumerate(thr(K0)):
                        scalar.activation(
                            junk_s[:, :], xs, AF.Relu, bias=-t, scale=1.0,
                            accum_out=acc_sb[:, base + 20 + j : base + 21 + j],
                        )
                scalar.wait_ge(vx_sem, 5)  # Sd ready
                scalar.activation(
                    junk_s[:, :], s_sb[:, 576:1152], AF.Ln,
                    accum_out=acc_sb[:, COL_LNSD : COL_LNSD + 1],
                )
                # spacer so the accumulator-write of the pass above has
                # retired before hist_sem releases the reduce matmul
                scalar.activation(junk_s[:, :], zero_sb[:, :], AF.Copy).then_inc(hist_sem)

            @block.vector
            def _(vector):
                vector.memset(acc_sb[:, :], 0.0)
                vector.memset(onesw_sb[:, :], 1.0)
                vector.memset(zero_sb[:, :], 0.0)
                vector.memset(onescol_sb[:, :], 1.0)
                vector.memset(partial_sb[:, :], 0.0)
                vector.wait_ge(dmaT_sem, 16)
                vector.wait_ge(dmaW_sem, 16)
                # n_c counts (need only targets)
                tfo = {"x0": 0, "x1": 1, "x2": 2}
                for (xk, c, K, base) in CLASSES:
                    h = tfo[xk]
                    vector.scalar_tensor_tensor(
                        junk_v[:, :], tf_sb[:, 576 * h : 576 * (h + 1)], float(c),
                        onesw_sb[:, :], OP.is_equal, OP.mult,
                        accum_out=acc_sb[:, base + 38 : base + 39],
                    )
                # ---- head1 softmax + x ----
                vector.wait_ge(cp_sem, CP_H1)
                vector.tensor_add(s_sb[:, 1152:1728], x1_sb[:, 0:576], x1_sb[:, 576:1152])
                vector.tensor_add(s_sb[:, 1152:1728], s_sb[:, 1152:1728], x1_sb[:, 1152:1728])
                vector.reciprocal(r_sb[:, 576:1152], s_sb[:, 1152:1728])
                ins = None
                for c in range(3):
                    cs = slice(576 * c, 576 * (c + 1))
                    vector.tensor_mul(x1_sb[:, cs], x1_sb[:, cs], r_sb[:, 576:1152])
                    ins = vector.scalar_tensor_tensor(
                        x1_sb[:, cs], tf_sb[:, 576:1152], float(c), x1_sb[:, cs],
                        OP.is_equal, OP.subtract,
                    )
                ins.then_inc(vx_sem)  # vx=1: x1
                # ---- head2 ----
                vector.wait_ge(cp_sem, CP_H2)
                vector.tensor_add(s_sb[:, 1728:2304], x2_sb[:, 0:576], x2_sb[:, 576:1152])
                vector.reciprocal(r_sb[:, 1152:1728], s_sb[:, 1728:2304])
                for c in range(2):
                    cs = slice(576 * c, 576 * (c + 1))
                    vector.tensor_mul(x2_sb[:, cs], x2_sb[:, cs], r_sb[:, 1152:1728])
                    ins = vector.scalar_tensor_tensor(
                        x2_sb[:, cs], tf_sb[:, 1152:1728], float(c), x2_sb[:, cs],
                        OP.is_equal, OP.subtract,
                    )
                ins.then_inc(vx_sem)  # vx=2: x2
                # ---- head0 ----
                vector.wait_ge(cp_sem, CP_H0)
                vector.tensor_add(s_sb[:, 0:576], x0_sb[:, 0:576], x0_sb[:, 576:1152])
                for c in range(2, 7):
                    ins = vector.tensor_add(
                        s_sb[:, 0:576], s_sb[:, 0:576], x0_sb[:, 576 * c : 576 * (c + 1)]
                    )
                ins.then_inc(vx_sem)  # vx=3: S0
                vector.reciprocal(r_sb[:, 0:576], s_sb[:, 0:576])
                for c in range(7):
                    cs = slice(576 * c, 576 * (c + 1))
                    vector.tensor_mul(x0_sb[:, cs], x0_sb[:, cs], r_sb[:, 0:576])
                    ins = vector.scalar_tensor_tensor(
                        x0_sb[:, cs], tf_sb[:, 0:576], float(c), x0_sb[:, cs],
                        OP.is_equal, OP.subtract,
                    )
                ins.then_inc(vx_sem)  # vx=4: x0
                # ---- dsn S ----
                vector.wait_ge(cp_sem, CP_HD)
                vector.tensor_add(s_sb[:, 576:1152], xd_sb[:, 0:576], xd_sb[:, 576:1152])
                for c in range(2, 7):
                    ins = vector.tensor_add(
                        s_sb[:, 576:1152], s_sb[:, 576:1152], xd_sb[:, 576 * c : 576 * (c + 1)]
                    )
                ins.then_inc(vx_sem)  # vx=5: Sd
                # CE z*fg accumulations
                for c in range(7):
                    vector.scalar_tensor_tensor(
                        junk_v[:, :], tf_sb[:, 0:576], float(c),
                        z0_sb[:, 576 * c : 576 * (c + 1)], OP.is_equal, OP.mult,
                        accum_out=acc_sb[:, COL_ZFG0 + c : COL_ZFG0 + c + 1],
                    )
                for c in range(7):
                    vector.scalar_tensor_tensor(
                        junk_v[:, :], tf_sb[:, 0:576], float(c),
                        zd_sb[:, 576 * c : 576 * (c + 1)], OP.is_equal, OP.mult,
                        accum_out=acc_sb[:, COL_ZFGD + c : COL_ZFGD + c + 1],
                    )
                # vector-owned hist: head1 TB (as -TB via min) + TF, head2 TF
                last = None
                for c in range(3):
                    base = 40 * (7 + c)
                    xs = x1_sb[:, 576 * c : 576 * (c + 1)]
                    for j, t in enumerate(thr(K12)):
                        # min(x + t, 0) accumulates -TB_j
                        last = vector.scalar_tensor_tensor(
                            junk_v[:, :], xs, t, zero_sb[:, :], OP.add, OP.min,
                            accum_out=acc_sb[:, base + j : base + j + 1],
                        )
                    for j, t in enumerate(thr(K12)):
                        last = vector.scalar_tensor_tensor(
                            junk_v[:, :], xs, t, zero_sb[:, :], OP.subtract, OP.max,
                            accum_out=acc_sb[:, base + 20 + j : base + 21 + j],
                        )
                for c in range(2):
                    base = 40 * (10 + c)
                    xs = x2_sb[:, 576 * c : 576 * (c + 1)]
                    for j, t in enumerate(thr(K12)):
                        last = vector.scalar_tensor_tensor(
                            junk_v[:, :], xs, t, zero_sb[:, :], OP.subtract, OP.max,
                            accum_out=acc_sb[:, base + 20 + j : base + 21 + j],
                        )
                vector.memset(junk_v[:, :], 0.0)  # spacer for accum writes
                vector.memset(junk_v[:, :], 0.0).then_inc(hist_sem)
                # ---- finale (interleaved to dodge small-tile RAW hazards) ----
                vector.wait_ge(mm_sem, 77)
                vector.tensor_copy(rst_sb[0:1, :], psR[0:1, :])
                vector.memset(junk_v[:, :], 0.0)  # spacer
                for i, (xk, c, K, base) in enumerate(CLASSES):
                    if xk == "x1":  # TB cols hold -TB: flip the difference
                        vector.tensor_sub(
                            tmpA_sb[0:1, 16 * i : 16 * i + K],
                            rst_sb[0:1, base + 1 : base + K + 1],
                            rst_sb[0:1, base : base + K],
                        )
                    else:
                        vector.tensor_sub(
                            tmpA_sb[0:1, 16 * i : 16 * i + K],
                            rst_sb[0:1, base : base + K],
                            rst_sb[0:1, base + 1 : base + K + 1],
                        )
                for i, (xk, c, K, base) in enumerate(CLASSES):
                    vector.tensor_scalar(
                        tmpB_sb[0:1, 16 * i : 16 * i + K],
                        tmpA_sb[0:1, 16 * i : 16 * i + K],
                        float(K), rst_sb[0:1, base + 38 : base + 39],
                        OP.mult, OP.add,
                    )
                for i, (xk, c, K, base) in enumerate(CLASSES):
                    vector.reciprocal(
                        tmpC_sb[0:1, 16 * i : 16 * i + K],
                        tmpB_sb[0:1, 16 * i : 16 * i + K],
                    )
                for i, (xk, c, K, base) in enumerate(CLASSES):
                    vector.tensor_sub(
                        tmpD_sb[0:1, 16 * i : 16 * i + K],
                        rst_sb[0:1, base + 20 : base + 20 + K],
                        rst_sb[0:1, base + 21 : base + 21 + K],
                    )
                for i, (xk, c, K, base) in enumerate(CLASSES):
                    vector.tensor_add(
                        tmpD_sb[0:1, 16 * i : 16 * i + K],
                        tmpD_sb[0:1, 16 * i : 16 * i + K],
                        tmpA_sb[0:1, 16 * i : 16 * i + K],
                    )
                for i, (xk, c, K, base) in enumerate(CLASSES):
                    vector.scalar_tensor_tensor(
                        tmpE_sb[0:1, 16 * i : 16 * i + K],
                        tmpD_sb[0:1, 16 * i : 16 * i + K],
                        1.0, tmpC_sb[0:1, 16 * i : 16 * i + K],
                        OP.mult, OP.mult,
                        accum_out=rst_sb[0:1, base + 39 : base + 40],
                    )
                vector.memset(junk_v[:, :], 0.0)  # spacer
                vector.tensor_mul(tmpw_sb[0:1, :], rst_sb[0:1, :], wv_sb[0:1, :])
                vector.memset(junk_v[:, :], 0.0)  # spacer
                vector.tensor_reduce(
                    partial_sb[0:1, 0:1], tmpw_sb[0:1, :],
                    mybir.AxisListType.X, OP.add,
                )
                vector.memset(junk_v[:, :], 0.0)  # spacer: partial write lands
                vector.memset(junk_v[:, :], 0.0).then_inc(fin_sem)

            @block.gpsimd
            def _(gpsimd):
                gpsimd.wait_ge(fin_sem, 1)
                gpsimd.dma_start(out=cc_in[:, :], in_=partial_sb[0:1, :]).then_inc(cdma_sem, 16)
                gpsimd.wait_ge(cdma_sem, 16)
                gpsimd.collective_compute(
                    "AllReduce", OP.add, replica_groups=[core_ids],
                    ins=[cc_in[:, :]], outs=[cc_out[:, :]],
                ).then_inc(cc_sem)
                gpsimd.wait_ge(cc_sem, 1)
                gpsimd.dma_start(out=p_out[:, :], in_=cc_out[:, :]).then_inc(cdma_sem, 16)
                gpsimd.wait_ge(cdma_sem, 32)
                if DEBUG:
                    n = 32
                    for name, t in [("d_x0", x0_sb), ("d_s", s_sb), ("d_acc", acc_sb), ("d_rst", rst_sb)]:
                        gpsimd.dma_start(out=dbg[name][:, :], in_=t[:, :]).then_inc(cdma_sem, 16)
                        n += 16
                        gpsimd.wait_ge(cdma_sem, n)

    return nc


# ---------------------------------------------------------------- host side --

def _interp_weights():
    s = np.linspace(np.float32(0.0), np.float32(95.0), 384).astype(np.float32)
    i0 = np.clip(np.floor(s).astype(np.int64), 0, 94)
    t = (s - i0).astype(np.float32)
    return i0, t


def _prep_core(inputs, core):
    b, half = core // 2, core % 2
    r0 = half * 192
    cy0 = 0 if half == 0 else 47
    i0, t = _interp_weights()

    uyt = np.zeros((49, 192), np.float32)
    for fy in range(192):
        f = r0 + fy
        uyt[i0[f] - cy0, fy] += np.float32(1.0) - t[f]
        uyt[i0[f] + 1 - cy0, fy] += t[f]

    ux = np.zeros((96, 384), np.float32)
    for X in range(384):
        ux[i0[X], X] += np.float32(1.0) - t[X]
        ux[i0[X] + 1, X] += t[X]

    # channel order: head1, head2, head0, dsn
    heads = [inputs["preds1"], inputs["preds2"], inputs["preds0"], inputs["preds_dsn"]]
    pa = np.zeros((49, NCH * 96), np.float32)
    idx = 0
    for arr in heads:
        for ch in range(arr.shape[1]):
            pa[:, idx * 96 : (idx + 1) * 96] = arr[b, ch, cy0 : cy0 + 49, :]
            idx += 1

    tg = np.zeros((128, 3 * 576), np.float32)
    for h, key in enumerate(["targets0", "targets1", "targets2"]):
        th = inputs[key][b, r0 : r0 + 192, :]  # [192, 384]
        tg[:, 576 * h : 576 * (h + 1)] = (
            th.reshape(192, 3, 128).transpose(2, 1, 0).reshape(128, 576)
        ).astype(np.float32)

    wv = np.zeros((1, ACC_W), np.float32)
    for (xk, c, K, base) in CLASSES:
        if xk == "x0":
            wv[0, base + 39] = (1.0 / 7.0) / 8.0
        elif xk == "x1":
            wv[0, base + 39] = (0.4 / 3.0) / 8.0
        else:
            wv[0, base + 39] = (0.4 / 2.0) / 8.0
    wv[0, COL_LNS0] = 1.0 / P_GLOBAL
    wv[0, COL_ZFG0 : COL_ZFG0 + 7] = -1.0 / P_GLOBAL
    wv[0, COL_LNSD] = 0.4 / P_GLOBAL
    wv[0, COL_ZFGD : COL_ZFGD + 7] = -0.4 / P_GLOBAL

    return {"preds_all": pa, "uyt": uyt, "ux": ux, "tgts": tg, "wvec": wv}


_NC_CACHE = None


def kernel(**inputs):
    global _NC_CACHE
    inputs = {k: np.asarray(v) for k, v in inputs.items()}
    if _NC_CACHE is None:
        _NC_CACHE = build_kernel()
    nc = _NC_CACHE
    in_maps = [_prep_core(inputs, core) for core in range(8)]
    res = run_bass_kernel_spmd(nc, in_maps, core_ids=list(range(8)))
    out = np.asarray(res.results[0]["out"], dtype=np.float32).reshape(-1)
    return np.asarray(out[0], dtype=np.float32)
